# revision 77
# baseline (speedup 1.0000x reference)
"""MoE layer (8 experts, top-2) on 8 TRN2 NeuronCores, expert-paired
half-F sharding, fp8 DoubleRow FFN with hi/lo error compensation.

Strategy:
  - Experts are paired big-with-small by token count; pair p maps to
    cores 2p (F columns 0:2048) and 2p+1 (F columns 2048:4096). Each
    core runs BOTH experts of its pair over half the FFN width, so the
    per-core token load is ~(C_big + C_small)/2 ~ mean instead of the
    max expert capacity. The two half-F partial y outputs of an expert
    are summed on host (the combine accumulation is already additive).
  - FFN matmuls run in fp8e4 (e4m3) with MatmulPerfMode.DoubleRow.
    mm1 uses the full 3-term scheme (x_hi w_hi + x_hi w_lo + x_lo w_hi,
    12 DR instrs per 128-col f-group). mm2 uses a partial 3-term: the
    h_lo w2_hi correction covers the first NCORRH (even tiles) or
    NCORRH-2 (odd tiles) of the KFH k-subtiles; err^2 ~ base^2 +
    (1-rho_mean)(2.65e-2)^2 with rho_mean=9.5/16 -> measured 1.79e-2
    vs the 2e-2 gate.
  - Weights are pre-scaled by 64 on host so fp8 values sit in e4m3's
    normal range; the 1/64 is folded into the gelu scale (mm1) and the
    combine weight (mm2).
  - The router (combine weight) is recomputed on device from the SAME
    fp8 hi/lo x stream the FFN uses, via 3-term logits against an fp8
    hi/lo router weight (pre-scaled by 64). Residual logit error ~1e-3
    absolute, negligible through softmax. This removes the fp32 x
    side-stream entirely. The renormalized top-2 weight is computed as
    sigmoid(l_mine - l_partner) = 0.5 + 0.5*tanh(./2): Tanh shares an
    act-function set with Gelu and Copy, so the whole kernel runs with
    zero mid-stream act-table reloads.
  - A small slot-A tail tile's mm2 runs in flipped orientation (output
    [h, tokens], cost proportional to the real token count instead of
    the 128-padded tile); its output leaves transposed and unscaled in
    yct, and the host applies the device-computed combine weight (wmo).
  - DMA: weights stream on the SP queue in consumption order (slot-B w2
    deferred into the loop so y stores drain between w1B and w2B); x
    chunks load via the ACT HWDGE queue. w1 is stored f-group-major per
    partition so every piece is a contiguous >=2KB strip (full DMA rate).
"""

from contextlib import ExitStack

import ml_dtypes
import numpy as np

P = 128
B, S, H, F, E = 2, 2048, 1024, 4096, 8
T = B * S            # 4096 tokens
KH = H // P          # 8   k-subtiles over H
F2 = F // 2          # 2048 f-columns per core
KFH = F2 // P        # 16  k-subtiles over F2
SCALE = 64.0
# mm2 partial error-correction: h_lo @ w2_hi only for the first NCORRH of
# the KFH subtiles (must be even).
NCORRH = 10

F8 = ml_dtypes.float8_e4m3

_CACHE = {}


def _chunks(C):
    out = []
    t0 = 0
    while t0 < C:
        size = min(512, C - t0)
        out.append((t0, size))
        t0 += size
    return out


def _build_nc(C1, C2, CL1, CL2, with_b2=False):
    import concourse.mybir as mybir
    import concourse.tile as tile
    from concourse import bacc

    dt = mybir.dt
    AF = mybir.ActivationFunctionType
    ALU = mybir.AluOpType
    AX = mybir.AxisListType
    PM = mybir.MatmulPerfMode

    Ct = C1 + C2
    TT1 = (CL1 + P - 1) // P
    TT2 = (CL2 + P - 1) // P
    TT = TT1 + TT2

    nc = bacc.Bacc(
        "TRN2", target_bir_lowering=False, debug=False, num_devices=E)

    xv = nc.declare_dram_parameter("xv", [2 * H, Ct], dt.float8e4, isOutput=False)
    rw8 = nc.declare_dram_parameter("rw8", [P, KH * 2 * E], dt.float8e4, isOutput=False)
    rw8h = nc.declare_dram_parameter("rw8h", [P, KH * E], dt.float8e4, isOutput=False)
    rbb = nc.declare_dram_parameter("rbb", [P, E], dt.float32, isOutput=False)
    selb = nc.declare_dram_parameter("selb", [P, 2 * E], dt.float32, isOutput=False)
    w1v = nc.declare_dram_parameter("w1v", [P, 2 * H * F // P], dt.float8e4,
                                    isOutput=False)
    w2v = nc.declare_dram_parameter("w2v", [2 * F, H], dt.float8e4, isOutput=False)
    b1d = nc.declare_dram_parameter("b1d", [P, 2 * KFH], dt.float32, isOutput=False)
    if with_b2:
        b2v = nc.declare_dram_parameter("b2v", [1, 4 * H], dt.float8e4, isOutput=False)
        onesd = nc.declare_dram_parameter("onesd", [1, 2 * P], dt.float8e4, isOutput=False)
    yc = nc.declare_dram_parameter("yc", [Ct, H], dt.bfloat16, isOutput=True)
    # Transposed, unscaled output for a small slot-A tail tile (see
    # emit_mm2_flip) + the combine-weight matrix for host-side scaling.
    yct = nc.declare_dram_parameter("yct", [H, P], dt.bfloat16, isOutput=True)
    yct_r = yct.rearrange("(t p) c -> p t c", p=P)
    wmo = nc.declare_dram_parameter("wmo", [P, TT], dt.float32, isOutput=True)

    xv_r = xv.rearrange("(s v p) t -> p s v t", v=2, p=P)
    rw8_r = rw8.rearrange("p (s v e) -> p s v e", v=2, e=E)
    rw8h_r = rw8h.rearrange("p (j v e) -> p j v e", v=2, e=E)
    # w1 is f-group-major per partition: [p, g, s, v, 128] — any g-range is a
    # contiguous >=2KB strip per partition, so every w1 piece DMAs at full
    # rate (f-sliced views of an [h, f] layout pay ~2x on 128B strips).
    w1_r = w1v.rearrange("p (g s v f) -> p g s v f", g=2 * KFH, s=KH, v=2)
    w2_r = w2v.rearrange("(s v p) h -> p s v h", v=2, p=P)

    # slot metadata: (token base in xv/yc, f-group/w2-subtile base, wmat tile
    # base, selb column base, real token width)
    slots = [
        dict(tb=0, fb=0, wb=0, sb=0, CL=CL1, cap=C1),
        dict(tb=C1, fb=KFH, wb=TT1, sb=E, CL=CL2, cap=C2),
    ]
    gchunks = []
    for sl in slots:
        for (t0, csz) in _chunks(sl["CL"]):
            gchunks.append((sl, t0, csz))

    with ExitStack() as ctx:
        tc = ctx.enter_context(tile.TileContext(nc))
        const = ctx.enter_context(tc.tile_pool(name="const", bufs=1))
        rpool = ctx.enter_context(tc.tile_pool(name="rtmp", bufs=3))
        rpsum = ctx.enter_context(tc.tile_pool(name="rpsum", bufs=2, space="PSUM"))
        xpool = ctx.enter_context(tc.tile_pool(name="xc", bufs=3))
        hbpool = ctx.enter_context(tc.tile_pool(name="hb", bufs=4))
        h1pool = ctx.enter_context(tc.tile_pool(name="h1", bufs=1))
        h1small = ctx.enter_context(tc.tile_pool(name="h1s", bufs=1))
        p1pool = ctx.enter_context(tc.tile_pool(name="p1", bufs=3, space="PSUM"))
        p2pool = ctx.enter_context(tc.tile_pool(name="p2", bufs=3, space="PSUM"))
        opool = ctx.enter_context(tc.tile_pool(name="ob", bufs=6))

        # Persistent tiles.
        rbb_s = const.tile([P, E], dt.float32)
        selb_s = const.tile([P, 2 * E], dt.float32)
        b1_s = const.tile([P, 2 * KFH], dt.float32)
        rw8_s = const.tile([P, KH, 2, E], dt.float8e4)
        rw8h_s = const.tile([P, KH // 2, 2, E], dt.float8e4)
        if with_b2:
            b2v_s = const.tile([1, 2, 2 * H], dt.float8e4)
            ones_s = const.tile([1, 2, P], dt.float8e4)
        wmat = const.tile([P, TT], dt.float32)

        w1_s = const.tile([P, 2 * KFH, KH, 2, P], dt.float8e4)
        w2_s = const.tile([P, 2 * KFH, 2, H], dt.float8e4)

        def load_xc(sl, t0, csz, halves=False):
            # Full-width loads (up to the slot's padded capacity): contiguous
            # per-partition runs DMA faster than a 508-col sliced AP, and the
            # router's 128-col tiles then never read unwritten SBUF.
            cap = sl["cap"]
            wl = min(512, cap - t0)
            xc = xpool.tile([P, KH, 2, 512], dt.float8e4, name="xc")
            src0 = sl["tb"] + t0
            if halves:
                nc.scalar.dma_start(xc[:, 0:4, :, 0:wl],
                                    xv_r[:, 0:4, :, src0:src0 + wl])
                nc.scalar.dma_start(xc[:, 4:8, :, 0:wl],
                                    xv_r[:, 4:8, :, src0:src0 + wl])
            else:
                nc.scalar.dma_start(xc[:, :, :, 0:wl],
                                    xv_r[:, :, :, src0:src0 + wl])
            return xc

        def load_w1(g0, g1):
            nc.sync.dma_start(w1_s[:, g0:g1], w1_r[:, g0:g1])

        # --- DMA streaming schedule, two queues ---
        # SP queue: weights (slot-A w1 f-major, slot-A w2 by H-halves, slot-B
        # w1 in startup; slot-B w2 deferred into the loop so the y output
        # stores — also on SP — drain between w1B and w2B instead of behind
        # the whole weight stream) plus the small consts.
        # DVE queue (HWDGE): all x chunk loads up-front, each with its own
        # buffer, so slot-B x never waits on the weight stream.
        sl0, t00, csz0 = gchunks[0]
        wl0 = min(512, sl0["cap"] - t00)
        xc0 = xpool.tile([P, KH, 2, 512], dt.float8e4, name="xc")
        # First pieces in exact fg0 consumption order: w1[g0,s<4] + x quarters
        # gate the very first matmuls; later subtiles stream behind them.
        # x chunk-0 halves ride the fast-issue SP queue; the two small w1[g0]
        # pieces go via ACT so both streams overlap from t=0.
        nc.scalar.dma_start(w1_s[:, 0, 0:4], w1_r[:, 0, 0:4])
        nc.sync.dma_start(xc0[:, 0:4, :, 0:wl0], xv_r[:, 0:4, :, t00:t00 + wl0])
        nc.scalar.dma_start(w1_s[:, 0, 4:8], w1_r[:, 0, 4:8])
        nc.sync.dma_start(xc0[:, 4:8, :, 0:wl0], xv_r[:, 4:8, :, t00:t00 + wl0])
        nc.sync.dma_start(b1_s[:], b1d[:])
        nc.sync.dma_start(rbb_s[:], rbb[:])
        nc.sync.dma_start(selb_s[:], selb[:])
        nc.sync.dma_start(rw8_s[:], rw8_r)
        nc.sync.dma_start(rw8h_s[:], rw8h_r)
        if with_b2:
            nc.sync.dma_start(b2v_s[:], b2v.rearrange("a (v h) -> a v h", v=2))
            nc.sync.dma_start(ones_s[:], onesd.rearrange("a (v p) -> a v p", v=2))
        load_w1(1, 2)
        load_w1(2, 3)
        load_w1(3, 4)
        load_w1(4, 6)
        load_w1(6, 8)
        load_w1(8, 12)
        load_w1(12, 16)
        # Chunk 1 loads on the SP queue here — after slot-A w1, so it cannot
        # steal DMA bandwidth from the stream PE is consuming; later chunks
        # issue at loop tops on the ACT queue.
        xc_pre = {0: xc0}
        if len(gchunks) > 1:
            sl1, t01, csz1 = gchunks[1]
            wl1 = min(512, sl1["cap"] - t01)
            xc1t = xpool.tile([P, KH, 2, 512], dt.float8e4, name="xc")
            s01 = sl1["tb"] + t01
            nc.sync.dma_start(xc1t[:, :, :, 0:wl1], xv_r[:, :, :, s01:s01 + wl1])
            xc_pre[1] = xc1t
        nc.sync.dma_start(w2_s[:, 0:KFH, :, 0:512], w2_r[:, 0:KFH, :, 0:512])
        nc.sync.dma_start(w2_s[:, 0:KFH, :, 512:1024], w2_r[:, 0:KFH, :, 512:1024])
        load_w1(16, 20)
        load_w1(20, 24)
        load_w1(24, 28)
        load_w1(28, 32)
        w2b_loaded = [False]

        def load_w2b():
            if w2b_loaded[0]:
                return
            w2b_loaded[0] = True
            nc.sync.dma_start(w2_s[:, KFH:2 * KFH, :, 0:512],
                              w2_r[:, KFH:2 * KFH, :, 0:512])
            nc.sync.dma_start(w2_s[:, KFH:2 * KFH, :, 512:1024],
                              w2_r[:, KFH:2 * KFH, :, 512:1024])

        def mm1_group(sl, xc, csz, h1, fl):
            ps1 = p1pool.tile([P, 512], dt.float32, name="ps1")[:, :csz]
            gf = sl["fb"] + fl
            # cross: pair dim = version: (w_hi, w_lo) x (x_lo, x_hi)
            # hi-hi: pair dim = adjacent k-subtiles
            first = True
            for half in range(2):
                for s in range(4 * half, 4 * half + 4):
                    nc.tensor.matmul(
                        ps1[:], w1_s[:, gf, s], xc[:, s, :, 0:csz],
                        start=first, stop=False, perf_mode=PM.DoubleRow)
                    first = False
                for j in range(2 * half, 2 * half + 2):
                    nc.tensor.matmul(
                        ps1[:], w1_s[:, gf, 2 * j:2 * j + 2, 0],
                        xc[:, 2 * j:2 * j + 2, 1, 0:csz],
                        start=False, stop=(j == KH // 2 - 1),
                        perf_mode=PM.DoubleRow)
            hb = hbpool.tile([P, 512], dt.bfloat16, name="hbf")[:, :csz]
            nc.scalar.activation(
                hb[:], ps1[:], AF.Gelu, bias=b1_s[:, gf:gf + 1],
                scale=1.0 / SCALE)
            # hi-copy alternates gpsimd/DVE: gpsimd alone (~1.3us/group) can't
            # quite keep up with PE's 1.28us/group; lo-subtract stays on DVE.
            eng = nc.gpsimd if fl % 2 == 0 else nc.vector
            eng.tensor_copy(h1[:, fl, 0, 0:csz], hb[:])
            if fl < NCORRH:
                nc.vector.tensor_tensor(h1[:, fl, 1, 0:csz], hb[:],
                                        h1[:, fl, 0, 0:csz], ALU.subtract)

        def alloc_h1(csz, small=False):
            # h1v: [P, s, (hi, lo), tokens] fp8; storage padded to 128 so
            # mm2 can read full token tiles
            pad = (csz + P - 1) // P * P
            if small:
                return h1small.tile([P, KFH, 2, 128], dt.float8e4, name="h1s")[:, :, :, :pad]
            return h1pool.tile([P, KFH, 2, 512], dt.float8e4, name="h1")[:, :, :, :pad]

        def emit_mm1(sl, xc, csz, small=False):
            h1 = alloc_h1(csz, small)
            for fl in range(KFH):
                mm1_group(sl, xc, csz, h1, fl)
            return h1

        def emit_router(sl, xc, t0, lc):
            # Fused per-tile router chain from the fp8 x stream. Placed at
            # chunk boundaries (between a chunk's mm1 and its mm2), where
            # Gelu<->Exp act-table reloads happen once per boundary and hide
            # under mm2. Logits are 3-term fp8 (SCALE-amplified; Exp divides
            # back). Reads up to 128 cols from xc; a partial tail tile reads
            # stale columns whose yc rows the host drops.
            gt = sl["wb"] + (t0 + lc) // P
            lg = rpsum.tile([P, E], dt.float32)
            for s in range(KH):
                nc.tensor.matmul(
                    lg[:], xc[:, s, :, lc:lc + P], rw8_s[:, s],
                    start=(s == 0), stop=False, perf_mode=PM.DoubleRow)
            for j in range(KH // 2):
                nc.tensor.matmul(
                    lg[:], xc[:, 2 * j:2 * j + 2, 1, lc:lc + P], rw8h_s[:, j],
                    start=False, stop=(j == KH // 2 - 1),
                    perf_mode=PM.DoubleRow)
            l = rpool.tile([P, E], dt.float32)
            nc.vector.tensor_tensor(l[:], lg[:], rbb_s[:], ALU.add)
            m1 = rpool.tile([P, 1], dt.float32)
            nc.vector.reduce_max(m1[:], l[:], axis=AX.X)
            ismax = rpool.tile([P, E], dt.float32)
            nc.vector.tensor_tensor(
                ismax[:], l[:], m1[:].to_broadcast((P, E)), ALU.is_equal)
            pen = rpool.tile([P, E], dt.float32)
            nc.vector.tensor_scalar_mul(pen[:], ismax[:], 1e30)
            lmask = rpool.tile([P, E], dt.float32)
            nc.vector.tensor_tensor(lmask[:], l[:], pen[:], ALU.subtract)
            m2 = rpool.tile([P, 1], dt.float32)
            nc.vector.reduce_max(m2[:], lmask[:], axis=AX.X)
            lsel = rpool.tile([P, E], dt.float32)
            nc.vector.tensor_tensor(
                lsel[:], l[:], selb_s[:, sl["sb"]:sl["sb"] + E], ALU.mult)
            lmine = rpool.tile([P, 1], dt.float32)
            nc.vector.reduce_sum(lmine[:], lsel[:], axis=AX.X)
            # Renormalized top-2 weight = sigmoid(l_mine - l_partner) where
            # the partner is the other top-2 logit: m2 if mine is the max,
            # else m1. sigmoid(x) = 0.5 + 0.5*tanh(x/2), and Tanh lives in
            # the same act-function set as Gelu and Copy — no table reloads.
            im = rpool.tile([P, 1], dt.float32)
            nc.vector.tensor_tensor(im[:], lmine[:], m1[:], ALU.is_ge)
            dm = rpool.tile([P, 1], dt.float32)
            nc.vector.tensor_tensor(dm[:], m2[:], m1[:], ALU.subtract)
            imdm = rpool.tile([P, 1], dt.float32)
            nc.vector.tensor_tensor(imdm[:], im[:], dm[:], ALU.mult)
            part = rpool.tile([P, 1], dt.float32)
            nc.vector.tensor_tensor(part[:], m1[:], imdm[:], ALU.add)
            dd = rpool.tile([P, 1], dt.float32)
            nc.vector.tensor_tensor(dd[:], lmine[:], part[:], ALU.subtract)
            th = rpool.tile([P, 1], dt.float32)
            nc.scalar.activation(th[:], dd[:], AF.Tanh, scale=1.0 / (2 * SCALE))
            # wmat = (0.5 + 0.5*tanh)/SCALE — the 1/SCALE folds the weight
            # pre-scaling into the combine weight.
            w0 = rpool.tile([P, 1], dt.float32)
            nc.vector.tensor_scalar_mul(w0[:], th[:], 0.5 / SCALE)
            nc.vector.tensor_scalar_add(wmat[:, gt:gt + 1], w0[:], 0.5 / SCALE)

        def boundary_router(sl, xc, t0, csz):
            for lc in range(0, csz, P):
                emit_router(sl, xc, t0, lc)

        def emit_mm2(sl, h1, t0, csz, group_cb=None, tail_split=False):
            # hh outer: the hh=0 groups run while the second w2 half streams.
            g = 0
            ntile = (csz + P - 1) // P
            sb = sl["fb"]
            for hh in range(H // 512):
                for ct in range(ntile):
                    gt = sl["wb"] + t0 // P + ct
                    row0 = sl["tb"] + t0 + ct * P
                    tsl = slice(ct * P, (ct + 1) * P)
                    last = (tail_split and hh == H // 512 - 1
                            and ct == ntile - 1)
                    # The very last group splits so its output pipeline
                    # (DVE+ACT+DMA) overlaps the later halves' matmuls; the
                    # split parts share one ob tile and one store (two
                    # stores would serialize ~625ns HWDGE issues in the
                    # end-of-kernel drain).
                    parts = [(0, 448), (448, 64)] if last else [(0, 512)]
                    obf = opool.tile([P, 512], dt.bfloat16,
                                     name="ob2") if last else None
                    for (o0, wid) in parts:
                        hsl = slice(hh * 512 + o0, hh * 512 + o0 + wid)
                        ps2 = p2pool.tile([P, 512], dt.float32, name="ps2")[:, :wid]
                        if with_b2:
                            # b2 bias folded into the accumulation: ones x b2
                            b2sl = slice(sl["fb"] // KFH * H + hsl.start,
                                         sl["fb"] // KFH * H + hsl.stop)
                            nc.tensor.matmul(
                                ps2[:], ones_s[:, :, 0:P], b2v_s[:, :, b2sl],
                                start=True, stop=False, perf_mode=PM.DoubleRow)
                        # hi-hi + hi-lo over all k; lo-hi correction only for
                        # the first NCORRH subtiles (partial 3-term).
                        for j in range(KFH // 2):
                            nc.tensor.matmul(
                                ps2[:], h1[:, 2 * j:2 * j + 2, 0, tsl],
                                w2_s[:, sb + 2 * j:sb + 2 * j + 2, 1, hsl],
                                start=(j == 0 and not with_b2), stop=False,
                                perf_mode=PM.DoubleRow)
                        for j in range(KFH // 2):
                            nc.tensor.matmul(
                                ps2[:], h1[:, 2 * j:2 * j + 2, 0, tsl],
                                w2_s[:, sb + 2 * j:sb + 2 * j + 2, 0, hsl],
                                start=False, stop=False,
                                perf_mode=PM.DoubleRow)
                        # Alternate 5/4 correction pairs by tile parity:
                        # mean dropped fraction 0.4375 -> err ~1.78e-2,
                        # saving one DR instr on every other group.
                        ncp = NCORRH // 2 - (gt % 2)
                        for j in range(ncp):
                            nc.tensor.matmul(
                                ps2[:], h1[:, 2 * j:2 * j + 2, 1, tsl],
                                w2_s[:, sb + 2 * j:sb + 2 * j + 2, 1, hsl],
                                start=False, stop=(j == ncp - 1),
                                perf_mode=PM.DoubleRow)
                        if last:
                            ob2 = obf[:, o0:o0 + wid]
                        else:
                            ob2 = opool.tile([P, 512], dt.bfloat16,
                                             name="ob2")[:, :wid]
                        nc.scalar.activation(
                            ob2[:], ps2[:], AF.Copy, bias=0.0,
                            scale=wmat[:, gt:gt + 1])
                        if not last:
                            nc.sync.dma_start(yc[row0:row0 + P, hsl], ob2[:])
                    if last:
                        fh = slice(hh * 512, (hh + 1) * 512)
                        nc.sync.dma_start(yc[row0:row0 + P, fh], obf[:])
                    if group_cb is not None:
                        group_cb(g)
                    g += 1

        def emit_mm2_flip(sl, h1, t0, csz):
            # Flipped-orientation mm2 for a small slot-A tail tile: output
            # [128h, csz] so PE cost scales with the real token count
            # (csz/512 of a standard group) instead of the 128-padded tile.
            # The combine weight (free dim here) is applied on host from
            # wmo; output goes to the transposed yct buffer in one store.
            sb = sl["fb"]
            obt = opool.tile([P, H // P, P], dt.bfloat16, name="obt")
            for ht in range(H // P):
                hsl = slice(ht * P, (ht + 1) * P)
                ps2 = p2pool.tile([P, 512], dt.float32, name="ps2")[:, :csz]
                first = True
                for j in range(KFH // 2):
                    nc.tensor.matmul(
                        ps2[:], w2_s[:, sb + 2 * j:sb + 2 * j + 2, 1, hsl],
                        h1[:, 2 * j:2 * j + 2, 0, 0:csz],
                        start=first, stop=False, perf_mode=PM.DoubleRow)
                    first = False
                for j in range(KFH // 2):
                    nc.tensor.matmul(
                        ps2[:], w2_s[:, sb + 2 * j:sb + 2 * j + 2, 0, hsl],
                        h1[:, 2 * j:2 * j + 2, 0, 0:csz],
                        start=False, stop=False, perf_mode=PM.DoubleRow)
                for j in range(NCORRH // 2):
                    nc.tensor.matmul(
                        ps2[:], w2_s[:, sb + 2 * j:sb + 2 * j + 2, 1, hsl],
                        h1[:, 2 * j:2 * j + 2, 1, 0:csz],
                        start=False, stop=(j == NCORRH // 2 - 1),
                        perf_mode=PM.DoubleRow)
                nc.scalar.activation(obt[:, ht, 0:csz], ps2[:], AF.Copy,
                                     bias=0.0, scale=1.0)
            nc.sync.dma_start(yct_r[:, :, 0:csz], obt[:, :, 0:csz])

        def emit_mm2_any(pend, tail_split=False):
            sl, h1, t0, csz = pend
            if csz <= P and sl is slots[0] and len(gchunks) > 1:
                emit_mm2_flip(sl, h1, t0, csz)
            else:
                emit_mm2(sl, h1, t0, csz, tail_split=tail_split)

        # Main pipeline over the global chunk list (slot A's chunks then
        # slot B's; the A->B transition behaves like any chunk boundary).
        h1_0 = emit_mm1(sl0, xc0, csz0)
        boundary_router(sl0, xc0, t00, csz0)
        pend = (sl0, h1_0, t00, csz0)
        for ci, (sl, t0, csz) in enumerate(gchunks[1:], start=1):
            if sl is slots[1]:
                load_w2b()
            xc = xc_pre[ci]
            if ci + 1 < len(gchunks) and ci + 1 not in xc_pre:
                xc_pre[ci + 1] = load_xc(*gchunks[ci + 1])
            if csz <= P:
                # Narrow chunk: its mm1 f-groups interleave into the previous
                # chunk's mm2 groups so the per-tile gelu/hi-lo-split latency
                # (which throttles PE on narrow tiles) hides under the wide
                # mm2 matmul groups. Needs a second (small) h1 buffer.
                h1n = alloc_h1(csz, small=True)
                state = {"f": 0}

                def il_cb(g, sl=sl, xc=xc, csz=csz, h1n=h1n, state=state):
                    while state["f"] < min(5 * (g + 1), KFH):
                        mm1_group(sl, xc, csz, h1n, state["f"])
                        state["f"] += 1

                emit_mm2(pend[0], pend[1], pend[2], pend[3], group_cb=il_cb)
                while state["f"] < KFH:
                    mm1_group(sl, xc, csz, h1n, state["f"])
                    state["f"] += 1
                pend = (sl, h1n, t0, csz)
            else:
                emit_mm2_any(pend)
                pend = (sl, emit_mm1(sl, xc, csz), t0, csz)
            boundary_router(sl, xc, t0, csz)
        # wmo rides the ACT queue at the end (host applies the tail scale).
        nc.scalar.dma_start(wmo[:], wmat[:])
        emit_mm2_any(pend, tail_split=True)
    return nc


def _get_nc(caps, cls_, with_b2=False):
    key = (caps, cls_, with_b2)
    if key not in _CACHE:
        nc = _build_nc(caps[0], caps[1], cls_[0], cls_[1], with_b2)
        nc.finalize()
        _CACHE[key] = nc
    return _CACHE[key]


def _split8(a):
    """a (f32) -> (hi, lo) fp8 pair with hi + lo ~= a."""
    hi = a.astype(F8)
    lo = (a - hi.astype(np.float32)).astype(F8)
    return hi, lo


def dispatch(hidden_states, router_w, router_b):
    """Host-side top-2 dispatch: per-expert token index lists + pairing."""
    x = np.asarray(hidden_states, dtype=np.float32).reshape(T, H)
    logits = x @ np.asarray(router_w, dtype=np.float32)
    logits = logits + np.asarray(router_b, dtype=np.float32)
    top2 = np.argpartition(logits, E - 2, axis=1)[:, E - 2:]  # [T, 2] unordered
    idx_lists = []
    for m in range(E):
        idx_lists.append(np.where((top2 == m).any(axis=1))[0])
    counts = np.array([len(ix) for ix in idx_lists])
    order = np.argsort(-counts, kind="stable")
    pairs = [(int(order[p]), int(order[E - 1 - p])) for p in range(E // 2)]
    CL1 = max(1, max(counts[a] for a, b in pairs))
    CL2 = max(1, max(counts[b] for a, b in pairs))
    C1 = max(P, (CL1 + P - 1) // P * P)
    C2 = max(P, (CL2 + P - 1) // P * P)
    return x, idx_lists, pairs, (C1, C2), (CL1, CL2)


def make_in_maps(hidden_states, router_w, router_b, w1, b1, w2, b2):
    x, idx_lists, pairs, caps, cls_ = dispatch(
        hidden_states, router_w, router_b)
    C1, C2 = caps
    xt = np.ascontiguousarray(x.T)            # [H, T] f32
    xhi, xlo = _split8(xt)
    rwS = np.asarray(router_w, dtype=np.float32) * SCALE
    rh, rl = _split8(rwS)                     # [H, E] fp8
    rh = rh.astype(F8); rl = rl.astype(F8)
    # rw8[p, s, (hi, lo), e]; rw8h[p, j, (s=2j, s=2j+1), e] of rw_hi
    rw8 = np.empty((P, KH, 2, E), dtype=F8)
    rw8h = np.empty((P, KH // 2, 2, E), dtype=F8)
    for s in range(KH):
        rw8[:, s, 0] = rh[s * P:(s + 1) * P]
        rw8[:, s, 1] = rl[s * P:(s + 1) * P]
    for j in range(KH // 2):
        rw8h[:, j, 0] = rh[(2 * j) * P:(2 * j + 1) * P]
        rw8h[:, j, 1] = rh[(2 * j + 1) * P:(2 * j + 2) * P]
    rbb = np.ascontiguousarray(np.broadcast_to(
        np.asarray(router_b, dtype=np.float32) * SCALE, (P, E)))
    w1 = np.asarray(w1, dtype=np.float32) * SCALE
    w2 = np.asarray(w2, dtype=np.float32) * SCALE
    b1 = np.asarray(b1, dtype=np.float32)
    b2 = np.asarray(b2, dtype=np.float32) * SCALE
    in_maps = []
    for core in range(E):
        p, phi = core // 2, core % 2
        a, b = pairs[p]
        fsl = slice(phi * F2, (phi + 1) * F2)
        pad = np.zeros(C1 + C2, dtype=np.int64)
        pad[:len(idx_lists[a])] = idx_lists[a]
        pad[C1:C1 + len(idx_lists[b])] = idx_lists[b]
        sel = np.zeros((P, 2 * E), dtype=np.float32)
        sel[:, a] = 1.0
        sel[:, E + b] = 1.0
        # xv: [s, (lo, hi), p, C1+C2]
        xg_lo = xlo[:, pad]
        xg_hi = xhi[:, pad]
        xvv = np.empty((KH, 2, P, C1 + C2), dtype=F8)
        xvv[:, 0] = xg_lo.reshape(KH, P, C1 + C2)
        xvv[:, 1] = xg_hi.reshape(KH, P, C1 + C2)
        # w1v: [p, g, s, (hi, lo), 128] f-group-major per partition so any
        # g-range is one contiguous strip per partition (full-rate DMA).
        # f-groups 0:KFH are expert a's half, KFH:2KFH b's.
        w1m = np.concatenate([w1[a][:, fsl], w1[b][:, fsl]], axis=1)
        w1hi, w1lo = _split8(w1m)
        G = 2 * KFH
        w1vv = np.empty((P, G, KH, 2, P), dtype=F8)
        w1vv[:, :, :, 0] = w1hi.reshape(KH, P, G, P).transpose(1, 2, 0, 3)
        w1vv[:, :, :, 1] = w1lo.reshape(KH, P, G, P).transpose(1, 2, 0, 3)
        # w2v: [s, (lo, hi), p, H] — subtiles 0:KFH expert a, KFH:2KFH b
        w2m = np.concatenate([w2[a][fsl, :], w2[b][fsl, :]], axis=0)
        w2hi, w2lo = _split8(w2m)
        w2vv = np.empty((2 * KFH, 2, P, H), dtype=F8)
        w2vv[:, 0] = w2lo.reshape(2 * KFH, P, H)
        w2vv[:, 1] = w2hi.reshape(2 * KFH, P, H)
        b1m = np.concatenate([b1[a][fsl], b1[b][fsl]])
        im = {}
        if np.any(b2):
            b2m = np.concatenate([b2[a], b2[b]])          # [2H]
            b2hi, b2lo = _split8(b2m)
            bv = np.empty((2, 2 * H), dtype=F8)
            bv[0] = b2hi
            bv[1] = b2lo
            im["b2v"] = bv.reshape(1, 4 * H)
            im["onesd"] = np.ones((1, 2 * P), dtype=F8)
        in_maps.append({
            **im,
            "xv": xvv.reshape(2 * H, C1 + C2),
            "rw8": rw8.reshape(P, KH * 2 * E),
            "rw8h": rw8h.reshape(P, KH * E),
            "rbb": rbb,
            "selb": sel,
            "w1v": np.ascontiguousarray(w1vv.reshape(P, 2 * H * F // P)),
            "w2v": w2vv.reshape(2 * F, H),
            "b1d": np.ascontiguousarray(b1m.reshape(2 * KFH, P).T),
        })
    return in_maps, idx_lists, pairs, caps, cls_


def run_device(in_maps, caps, cls_, with_b2=False):
    from concourse.bass_utils import run_bass_kernel_spmd

    nc = _get_nc(caps, cls_, with_b2=with_b2)
    res = run_bass_kernel_spmd(nc, in_maps, core_ids=list(range(E)))
    return res.results


def kernel(hidden_states, router_w, router_b, w1, b1, w2, b2):
    in_maps, idx_lists, pairs, caps, cls_ = make_in_maps(
        hidden_states, router_w, router_b, w1, b1, w2, b2)
    C1, C2 = caps
    with_b2 = bool(np.any(np.asarray(b2)))
    # One retry guards against a rare transient execution glitch observed on
    # the very first load of a freshly compiled NEFF (garbage ~1e35 values);
    # a healthy output has absmax of a few units.
    last_err = None
    for attempt in range(3):
        try:
            results = run_device(in_maps, caps, cls_, with_b2)
        except Exception as e:  # transient NRT/axon failures observed
            last_err = e
            import time as _time
            _time.sleep(10)
            continue
        CL1, CL2 = cls_
        chunksA = _chunks(CL1)
        t0l, cszl = chunksA[-1]
        flip = cszl <= P and (len(chunksA) + len(_chunks(CL2))) > 1
        gt_tail = (CL1 + P - 1) // P - 1
        acc = np.zeros((T, H), dtype=np.float32)
        for core in range(E):
            a, b = pairs[core // 2]
            ixa, ixb = idx_lists[a], idx_lists[b]
            na = len(ixa)
            yb = np.asarray(results[core]["yc"], dtype=np.float32)
            if flip:
                nhead = min(na, t0l)
                acc[ixa[:nhead]] += yb[:nhead]
                if na > t0l:
                    # Tail rows come transposed and unscaled; apply the
                    # device-computed combine weight here.
                    wv = np.asarray(results[core]["wmo"],
                                    dtype=np.float32)[:, gt_tail]
                    yt = np.asarray(results[core]["yct"], dtype=np.float32)
                    nt = na - t0l
                    acc[ixa[t0l:na]] += yt[:, :nt].T * wv[:nt, None]
            else:
                acc[ixa] += yb[:na]
            acc[ixb] += yb[C1:C1 + len(ixb)]
        if np.isfinite(acc).all() and np.abs(acc).max() < 1e4:
            return acc.reshape(B, S, H)
    if last_err is not None:
        raise last_err
    return acc.reshape(B, S, H)
